# revision 24
# baseline (speedup 1.0000x reference)
"""Trainium2 Bass kernel for nn_Comm_OUT (Linear+BN+PReLU -> 20-step GRU ->
BN+PReLU -> 1x1 conv -> BN+PReLU -> linear head), data-parallel over 8 cores.

v3 vs the bf16 baseline:
- The 20x recurrent W_hh gate matmuls run in fp8-e4m3 DoubleRow mode (K=256
  in one PE pass, half the streaming cycles). Weights ship x64, h is cast
  x8 to fp8 each step; the x512 psum scale is absorbed by the free affine
  of the sigmoid/tanh activations (gi is pre-scaled x512 on chip).
- PReLU2 is linearized: prelu2(y) = a2*y + (1-a2)*relu(y). The a2 branch
  folds into the conv weights (consuming h directly); the relu branch is a
  single DVE tensor_scalar max(s2*h, -t2) per half. This replaces the
  3-op DVE prelu chain of the baseline.
- Step 0 keeps the PE warm by running the gate matmuls against a zeroed h8
  (results are exact) while skipping the dead elementwise ops.
Conv / head / stage-1 matmuls stay bf16: fp8 there costs 3-4% max-err
(measured), far over the 2e-2 budget, while gates-only fp8 sits at ~1.4%.
"""

import numpy as np
import ml_dtypes

import concourse.bacc as bacc
import concourse.mybir as mybir
import concourse.tile as tile
from concourse import bass_utils

AF = mybir.ActivationFunctionType
OP = mybir.AluOpType
PM = mybir.MatmulPerfMode
F32 = mybir.dt.float32
BF16 = mybir.dt.bfloat16
FP8 = mybir.dt.float8e4
U8 = mybir.dt.uint8

E, S, F, H, C, L = 64, 128, 640, 256, 32, 20
EPS = 1e-5
NCORES = 8
B = E * S              # 8192
BC = B // NCORES       # 1024 batch rows per core
NCH = 2                # chunks per core
CB = BC // NCH         # 512 batch rows per chunk (PSUM-bank friendly)

SH = 8.0               # h prescale into fp8 (keeps |8h|>=0.016 out of the
                       # e4m3 subnormal range the PE flushes to zero)
SW = 64.0              # W_hh prescale into fp8
SP = SH * SW           # gate-psum scale (512)

# const-vector columns (packed [128, NV], one column per per-partition vec)
CV_S1, CV_T1 = 0, 2
CV_GIBRZ = 4           # 512*(b_ih+b_hh) r,z (4 cols)
CV_GIBN = 8            # 512*b_ih n-part (2 cols)
CV_S2Z, CV_T2Z = 10, 12  # s2, -t2 (or t2) per m (2 each)
CV_C3 = 14             # conv const per m (2 cols)
CV_BMU, CV_A1, CV_A3 = 16, 17, 18
CV_BHN = 19            # SP*b_hh n-part (2 cols, zero in practice)
NV = 21

# bf16 weight pack (wp) column layout
WB_LIN = 0             # W_lin.T k-major: 5*256 = 1280
WB_IH = 1280           # W_ih.T k-major: 2*768 = 1536
WB_W1 = 2816           # conv a2-branch  (k-major): 2*256 = 512
WB_W2 = 3328           # conv relu-branch (k-major): 2*256 = 512
WB_MU = 3840           # W_mu.T k-major: 2*32 = 64
WB_ID = 3904           # identity 128
WB_COLS = 4032

_CACHE: dict = {}


def build_program(bhn_zero=True, zr_1op=True):
    nc = bacc.Bacc("TRN2", target_bir_lowering=False, debug=False)

    xT_h = nc.dram_tensor("xT", [128, 5 * BC], BF16, kind="ExternalInput")
    wp_h = nc.dram_tensor("wp", [128, WB_COLS], BF16, kind="ExternalInput")
    # fp8 weights ship as raw bytes; bitcast to fp8 at the use site
    wp8_h = nc.dram_tensor("wp8", [128, 1536], U8, kind="ExternalInput")
    cv_h = nc.dram_tensor("cv", [128, NV], F32, kind="ExternalInput")
    out_h = nc.dram_tensor("out", [BC, L * C], F32, kind="ExternalOutput")

    with tile.TileContext(nc) as tc:
        with (
            tc.tile_pool(name="consts", bufs=1) as cpool,
            tc.tile_pool(name="gi", bufs=1) as gip,
            tc.tile_pool(name="hp", bufs=2) as hp,
            tc.tile_pool(name="gates", bufs=3) as gp,
            tc.tile_pool(name="s24", bufs=2) as sp,
            tc.tile_pool(name="ps2", bufs=3, space="PSUM") as ps2,
            tc.tile_pool(name="ps1", bufs=2, space="PSUM") as ps1,
        ):
            cvt = cpool.tile([128, NV], F32, tag="cv")
            nc.sync.dma_start(cvt[:], cv_h[:])
            wpt_a = cpool.tile([128, WB_W1], BF16, tag="wpta")
            nc.sync.dma_start(wpt_a[:], wp_h[:, :WB_W1])
            wpt_b = cpool.tile([128, WB_COLS - WB_W1], BF16, tag="wptb")
            wp8t_u8 = cpool.tile([128, 1536], U8, tag="wp8t")
            wlin_t = wpt_a[:, WB_LIN : WB_LIN + 1280]
            wih_t = wpt_a[:, WB_IH : WB_IH + 1536]
            w1_t = wpt_b[:, WB_W1 - WB_W1 : WB_W2 - WB_W1]
            w2_t = wpt_b[:, WB_W2 - WB_W1 : WB_MU - WB_W1]
            wmu_t = wpt_b[:, WB_MU - WB_W1 : WB_ID - WB_W1]
            idtb = wpt_b[:, WB_ID - WB_W1 : WB_COLS - WB_W1]
            whh_dr = wp8t_u8[:, :].bitcast(FP8).rearrange("p (k r) -> p k r", k=2)

            def pp(col):
                return cvt[:, col : col + 1]

            def drmov(tile_):  # DR moving view of a [128, 2*CB] fp8 tile
                return tile_[:].rearrange("p (k b) -> p k b", k=2)

            # ---- stage 1: x1 = prelu(bn(x @ W_lin.T)); gi = x1 @ W_ih.T ----
            # all gi parts stored x512 (r/z feed PSUM preloads; tanh's free
            # scale absorbs 1/512 for the n part)
            girz = [None] * NCH
            gin = [None] * NCH
            with tc.tile_pool(name="stage1", bufs=1) as xp:
                xts = []
                for k in range(5):
                    xk = xp.tile([128, BC], BF16, tag=f"xT{k}", name=f"xT{k}")
                    nc.sync.dma_start(xk[:], xT_h[:, k * BC : (k + 1) * BC])
                    xts.append(xk)
                nc.sync.dma_start(wpt_b[:], wp_h[:, WB_W1:])
                nc.sync.dma_start(wp8t_u8[:], wp8_h[:])
                for c in range(NCH):
                    ps_x1 = ps2.tile([128, 2 * CB], F32, tag="g")
                    for k in range(5):
                        for m in range(2):
                            nc.tensor.matmul(
                                ps_x1[:, m * CB : (m + 1) * CB],
                                wlin_t[:, k * H + m * 128 : k * H + (m + 1) * 128],
                                xts[k][:, c * CB : (c + 1) * CB],
                                start=(k == 0),
                                stop=(k == 4),
                            )
                    x1p = xp.tile([128, 2 * CB], BF16, tag=f"x1p{c}")
                    for m in range(2):
                        nc.scalar.activation(
                            x1p[:, m * CB : (m + 1) * CB],
                            ps_x1[:, m * CB : (m + 1) * CB],
                            AF.Prelu,
                            bias=pp(CV_T1 + m),
                            scale=pp(CV_S1 + m),
                            alpha=pp(CV_A1),
                        )
                    girz[c] = gip.tile(
                        [128, 4 * CB], BF16, tag=f"girz{c}", name=f"girz{c}"
                    )
                    gin[c] = gip.tile(
                        [128, 2 * CB], BF16, tag=f"gin{c}", name=f"gin{c}"
                    )
                    for m in range(6):
                        ps_gi = ps1.tile([128, CB], F32, tag="p")
                        for k in range(2):
                            nc.tensor.matmul(
                                ps_gi[:],
                                wih_t[:, k * 3 * H + m * 128 : k * 3 * H + (m + 1) * 128],
                                x1p[:, k * CB : (k + 1) * CB],
                                start=(k == 0),
                                stop=(k == 1),
                            )
                        dstv = (
                            girz[c][:, m * CB : (m + 1) * CB]
                            if m < 4
                            else gin[c][:, (m - 4) * CB : (m - 3) * CB]
                        )
                        nc.scalar.activation(
                            dstv,
                            ps_gi[:],
                            AF.Identity,
                            bias=pp((CV_GIBRZ + m) if m < 4 else (CV_GIBN + m - 4)),
                            scale=SP,
                        )

            hs = [None] * NCH       # h (bf16)
            h8s = [None] * NCH      # 8*h (fp8) for the DR gate matmuls
            zrs = [None] * NCH      # relu-branch conv operand (bf16)
            y2s = [None] * NCH
            packs = [None] * NCH

            # step-0 h8 is just zeros; running the gate matmuls against it
            # keeps the PE warm and the results exact
            for c in range(NCH):
                h80 = hp.tile([128, 2 * CB], FP8, tag=f"h8{c}", name=f"h8{c}")
                nc.vector.memset(h80[:], 0.0)
                h8s[c] = h80

            def emit_stage2(c):
                h8 = hp.tile([128, 2 * CB], FP8, tag=f"h8{c}", name=f"h8{c}")
                nc.gpsimd.tensor_scalar_mul(h8[:], hs[c][:], SH)
                h8s[c] = h8
                zr = gp.tile([128, 2 * CB], BF16, tag="zr", name=f"zr{c}")
                for m in range(2):
                    nc.gpsimd.tensor_scalar(
                        zr[:, m * CB : (m + 1) * CB],
                        hs[c][:, m * CB : (m + 1) * CB],
                        pp(CV_S2Z + m),
                        pp(CV_T2Z + m),
                        op0=OP.mult,
                        op1=OP.max if zr_1op else OP.add,
                    )
                if not zr_1op:
                    nc.gpsimd.tensor_scalar_max(zr[:], zr[:], 0.0)
                zrs[c] = zr

            def emit_conv(c):
                """conv psum for the latest h: W1@h + W2@zr (c3 rides the
                prelu3 bias)"""
                ps_cv = ps2.tile([128, 2 * CB], F32, tag="g", name=f"pscv{c}")
                for m in range(2):
                    for k in range(2):
                        nc.tensor.matmul(
                            ps_cv[:, m * CB : (m + 1) * CB],
                            w1_t[:, k * H + m * 128 : k * H + (m + 1) * 128],
                            hs[c][:, k * CB : (k + 1) * CB],
                            start=(k == 0),
                            stop=False,
                        )
                        nc.tensor.matmul(
                            ps_cv[:, m * CB : (m + 1) * CB],
                            w2_t[:, k * H + m * 128 : k * H + (m + 1) * 128],
                            zrs[c][:, k * CB : (k + 1) * CB],
                            start=False,
                            stop=(k == 1),
                        )
                return ps_cv

            def emit_prelu3(ps_cv, c):
                y2 = gp.tile([128, 2 * CB], BF16, tag="y2", name=f"y2_{c}")
                for m in range(2):
                    nc.scalar.activation(
                        y2[:, m * CB : (m + 1) * CB],
                        ps_cv[:, m * CB : (m + 1) * CB],
                        AF.Prelu,
                        bias=pp(CV_C3 + m),
                        alpha=pp(CV_A3),
                    )
                y2s[c] = y2

            def emit_wmu(t, c):
                j = t % 4
                if j == 0:
                    packs[c] = ps1.tile([128, CB], F32, tag="p", name=f"pack{c}")
                for k in range(2):
                    nc.tensor.matmul(
                        packs[c][32 * j : 32 * (j + 1), :],
                        wmu_t[:, k * C : (k + 1) * C],
                        y2s[c][:, k * CB : (k + 1) * CB],
                        start=(k == 0),
                        stop=(k == 1),
                        tile_position=(0, 32 * j),
                    )

            def emit_pack_out(t, c):
                if t % 4 != 3 or packs[c] is None:
                    return
                ps_pack = packs[c]
                pk = sp.tile([128, CB], F32, tag="pk", name=f"pk{c}")
                nc.scalar.activation(pk[:], ps_pack[:], AF.Identity, bias=pp(CV_BMU))
                tr = sp.tile([128, CB], F32, tag="tr", name=f"tr{c}")
                nc.vector.transpose(tr[:], pk[:])
                l4 = t // 4
                dst = out_h[:].rearrange(
                    "(c bh bl) (l4 li cc) -> c l4 li bl bh cc",
                    c=NCH, bh=CB // 32, bl=32, l4=L // 4, li=4, cc=C,
                )
                for li in range(4):
                    nc.sync.dma_start(
                        dst[c, l4, li],
                        tr[32 * li : 32 * (li + 1), :].rearrange(
                            "p (bh cc) -> p bh cc", bh=CB // 32
                        ),
                    )

            def emit_gate_mms_chunk(c, psr, psz, psn):
                for g, ps in ((0, psr), (1, psz)):
                    for m in range(2):
                        nc.tensor.matmul(
                            ps[:, m * CB : (m + 1) * CB],
                            idtb,
                            girz[c][:, (2 * g + m) * CB : (2 * g + m + 1) * CB],
                            start=True,
                            stop=False,
                        )
                for g, ps in ((0, psr), (1, psz), (2, psn)):
                    for m in range(2):
                        row = 2 * g + m
                        nc.tensor.matmul(
                            ps[:, m * CB : (m + 1) * CB],
                            whh_dr[:, :, row * 128 : (row + 1) * 128],
                            drmov(h8s[c]),
                            start=(g == 2),
                            stop=True,
                            perf_mode=PM.DoubleRow,
                        )

            # ---- GRU steps t = 0..L-1 (uniform; t=0 skips dead elementwise) ----
            for t in range(L):
                psr, psz, psn = [], [], []
                for c in range(NCH):
                    psr.append(ps2.tile([128, 2 * CB], F32, tag="g", name=f"psr{c}"))
                    psz.append(ps2.tile([128, 2 * CB], F32, tag="g", name=f"psz{c}"))
                    psn.append(ps2.tile([128, 2 * CB], F32, tag="g", name=f"psn{c}"))
                for c in range(NCH):
                    emit_gate_mms_chunk(c, psr[c], psz[c], psn[c])
                pscs = [None] * NCH
                if t >= 1:
                    for c in range(NCH):
                        pscs[c] = emit_conv(c)      # conv of h_t (prev state)
                if t >= 2:
                    for c in range(NCH):
                        emit_wmu(t - 2, c)
                for c in range(NCH):
                    z_sb = gp.tile([128, 2 * CB], BF16, tag="z", name=f"z{c}")
                    nc.scalar.activation(z_sb[:], psz[c][:], AF.Sigmoid, scale=1.0 / SP)
                    if t == 0:
                        # psn = W@0 = 0 -> n = tanh(gin); h1 = n - z*n
                        n_sb = gp.tile([128, 2 * CB], BF16, tag="n", name=f"n{c}")
                        nc.scalar.activation(n_sb[:], gin[c][:], AF.Tanh, scale=1.0 / SP)
                        v_sb = gp.tile([128, 2 * CB], BF16, tag="v", name=f"v{c}")
                        nc.vector.tensor_tensor(v_sb[:], z_sb[:], n_sb[:], OP.mult)
                        h = hp.tile([128, 2 * CB], BF16, tag=f"h{c}", name=f"h{c}")
                        nc.vector.tensor_tensor(h[:], n_sb[:], v_sb[:], OP.subtract)
                        hs[c] = h
                        continue
                    r_sb = gp.tile([128, 2 * CB], BF16, tag="r", name=f"r{c}")
                    nc.scalar.activation(r_sb[:], psr[c][:], AF.Sigmoid, scale=1.0 / SP)
                    t_sb = gp.tile([128, 2 * CB], BF16, tag="t", name=f"t{c}")
                    nc.vector.tensor_tensor(t_sb[:], psn[c][:], r_sb[:], OP.mult)
                    a_sb = gp.tile([128, 2 * CB], BF16, tag="a", name=f"a{c}")
                    if bhn_zero:
                        nc.vector.tensor_tensor(a_sb[:], t_sb[:], gin[c][:], OP.add)
                    else:
                        # a*SP = t + gin + SP*bhn*r
                        tmp = gp.tile([128, 2 * CB], BF16, tag="tb", name=f"tb{c}")
                        for m in range(2):
                            nc.vector.tensor_scalar_mul(
                                tmp[:, m * CB : (m + 1) * CB],
                                r_sb[:, m * CB : (m + 1) * CB],
                                pp(CV_BHN + m),
                            )
                        nc.vector.tensor_tensor(tmp[:], t_sb[:], tmp[:], OP.add)
                        nc.vector.tensor_tensor(a_sb[:], tmp[:], gin[c][:], OP.add)
                    n_sb = gp.tile([128, 2 * CB], BF16, tag="n", name=f"n{c}")
                    nc.scalar.activation(n_sb[:], a_sb[:], AF.Tanh, scale=1.0 / SP)
                    u_sb = gp.tile([128, 2 * CB], BF16, tag="u", name=f"u{c}")
                    nc.vector.tensor_tensor(u_sb[:], hs[c][:], n_sb[:], OP.subtract)
                    v_sb = gp.tile([128, 2 * CB], BF16, tag="v", name=f"v{c}")
                    nc.vector.tensor_tensor(v_sb[:], z_sb[:], u_sb[:], OP.mult)
                    h = hp.tile([128, 2 * CB], BF16, tag=f"h{c}", name=f"h{c}")
                    nc.vector.tensor_tensor(h[:], n_sb[:], v_sb[:], OP.add)
                    hs[c] = h
                    if t >= 1 and pscs[c] is not None:
                        emit_prelu3(pscs[c], c)
                for c in range(NCH):
                    emit_stage2(c)
                if t >= 2:
                    for c in range(NCH):
                        emit_pack_out(t - 2, c)
            # epilogue: conv/head for the last two states
            pscs = [emit_conv(c) for c in range(NCH)]
            for c in range(NCH):
                emit_wmu(L - 2, c)
            for c in range(NCH):
                emit_prelu3(pscs[c], c)
            for c in range(NCH):
                emit_pack_out(L - 2, c)
            for c in range(NCH):
                emit_wmu(L - 1, c)
            for c in range(NCH):
                emit_pack_out(L - 1, c)

    nc.compile()
    return nc


def _to_fp8(a):
    return np.clip(np.asarray(a, np.float32), -240.0, 240.0).astype(
        ml_dtypes.float8_e4m3
    )


def _prep_inputs(inputs):
    f32 = np.float32
    bf = ml_dtypes.bfloat16
    x = np.ascontiguousarray(np.asarray(inputs["h_w_action"], f32).reshape(B, F))
    W_lin = np.asarray(inputs["W_lin"], f32)
    b_lin = np.asarray(inputs["b_lin"], f32)
    W_ih = np.asarray(inputs["W_ih"], f32)
    W_hh = np.asarray(inputs["W_hh"], f32)
    b_ih = np.asarray(inputs["b_ih"], f32)
    b_hh = np.asarray(inputs["b_hh"], f32)
    Wc = np.asarray(inputs["Wc"], f32)
    bc = np.asarray(inputs["bc"], f32)
    W_mu = np.asarray(inputs["W_mu"], f32)
    b_mu = np.asarray(inputs["b_mu"], f32)
    a1 = f32(np.asarray(inputs["a1"]).reshape(-1)[0])
    a2 = f32(np.asarray(inputs["a2"]).reshape(-1)[0])
    a3 = f32(np.asarray(inputs["a3"]).reshape(-1)[0])

    def bnfold(g, beta, m, v):
        s = np.asarray(g, f32) / np.sqrt(np.asarray(v, f32) + EPS)
        return s, np.asarray(beta, f32) - np.asarray(m, f32) * s

    s1, t1 = bnfold(inputs["g1"], inputs["beta1"], inputs["m1"], inputs["v1"])
    s2, t2 = bnfold(inputs["g2"], inputs["beta2"], inputs["m2"], inputs["v2"])
    s3, t3 = bnfold(inputs["g3"], inputs["beta3"], inputs["m3"], inputs["v3"])
    t1 = t1 + s1 * b_lin
    t3 = t3 + s3 * bc
    gib = b_ih.copy()
    gib[: 2 * H] += b_hh[: 2 * H]
    bhn = b_hh[2 * H :]
    bhn_zero = bool(np.all(bhn == 0))
    zr_1op = bool(np.all(s2 > 0))

    # conv folding: conv_in = a2*s2*h + (1-a2)*zr + const
    #   zr_1op: zr = max(s2*h, -t2), const = t2
    #   else:   zr = relu(s2*h + t2), const = a2*t2
    W1 = a2 * (s3[:, None] * Wc * s2[None, :])
    W2 = (1.0 - a2) * (s3[:, None] * Wc)
    c3 = s3 * (Wc @ (t2 if zr_1op else a2 * t2)) + t3

    cv = np.zeros((128, NV), f32)
    for col, vec in ((CV_S1, s1), (CV_T1, t1)):
        cv[:, col] = vec[:128]
        cv[:, col + 1] = vec[128:]
    for m in range(4):
        cv[:, CV_GIBRZ + m] = SP * gib[m * 128 : (m + 1) * 128]
    for m in range(2):
        cv[:, CV_GIBN + m] = SP * gib[(4 + m) * 128 : (5 + m) * 128]
        cv[:, CV_BHN + m] = SP * bhn[m * 128 : (m + 1) * 128]
        cv[:, CV_S2Z + m] = s2[m * 128 : (m + 1) * 128]
        cv[:, CV_T2Z + m] = (-t2 if zr_1op else t2)[m * 128 : (m + 1) * 128]
        cv[:, CV_C3 + m] = c3[m * 128 : (m + 1) * 128]
    cv[:, CV_BMU] = np.tile(b_mu, 4)
    cv[:, CV_A1] = a1
    cv[:, CV_A3] = a3

    def kmaj(a, kt):  # [kt*128, n] -> [128, kt*n] k-tile-major columns
        n = a.shape[1]
        return a.reshape(kt, 128, n).transpose(1, 0, 2).reshape(128, kt * n)

    wp = np.concatenate(
        [
            kmaj(np.ascontiguousarray(W_lin.T), 5),
            kmaj(np.ascontiguousarray(W_ih.T), 2),
            kmaj(np.ascontiguousarray(W1.T), 2),
            kmaj(np.ascontiguousarray(W2.T), 2),
            kmaj(np.ascontiguousarray(W_mu.T), 2),
            np.eye(128, dtype=f32),
        ],
        axis=1,
    ).astype(bf)
    wp8 = kmaj(np.ascontiguousarray(SW * W_hh.T), 2)
    shared = {
        "wp": np.ascontiguousarray(wp),
        "wp8": np.ascontiguousarray(_to_fp8(wp8)).view(np.uint8),
        "cv": cv,
    }
    in_maps = []
    for i in range(NCORES):
        m = dict(shared)
        xtc = np.ascontiguousarray(x[i * BC : (i + 1) * BC, :].T)  # [640, BC]
        m["xT"] = np.ascontiguousarray(kmaj(xtc, 5).astype(bf))
        in_maps.append(m)
    return in_maps, bhn_zero, zr_1op


def kernel(**inputs) -> np.ndarray:
    in_maps, bhn_zero, zr_1op = _prep_inputs(inputs)
    key = ("nc", bhn_zero, zr_1op)
    if key not in _CACHE:
        _CACHE[key] = build_program(bhn_zero, zr_1op)
    nc = _CACHE[key]
    _CACHE["last"] = nc
    res = bass_utils.run_bass_kernel_spmd(nc, in_maps, core_ids=list(range(NCORES)))
    outs = [np.asarray(r["out"], np.float32) for r in res.results]
    return np.concatenate(outs, axis=0).reshape(E, S, L, C)


# revision 25
# speedup vs baseline: 4.9305x; 4.9305x over previous
"""Trainium2 Bass kernel for nn_Comm_OUT (Linear+BN+PReLU -> 20-step GRU ->
BN+PReLU -> 1x1 conv -> BN+PReLU -> linear head), data-parallel over 8 cores.

v3 vs the bf16 baseline:
- The 20x recurrent W_hh gate matmuls run in fp8-e4m3 DoubleRow mode (K=256
  in one PE pass, half the streaming cycles). Weights ship x64, h is cast
  x8 to fp8 each step; the x512 psum scale is absorbed by the free affine
  of the sigmoid/tanh activations (gi is pre-scaled x512 on chip).
- PReLU2 is linearized: prelu2(y) = a2*y + (1-a2)*relu(y). The a2 branch
  folds into the conv weights (consuming h directly); the relu branch is a
  single DVE tensor_scalar max(s2*h, -t2) per half. This replaces the
  3-op DVE prelu chain of the baseline.
- Step 0 keeps the PE warm by running the gate matmuls against a zeroed h8
  (results are exact) while skipping the dead elementwise ops.
Conv / head / stage-1 matmuls stay bf16: fp8 there costs 3-4% max-err
(measured), far over the 2e-2 budget, while gates-only fp8 sits at ~1.4%.
"""

import numpy as np
import ml_dtypes

import concourse.bacc as bacc
import concourse.mybir as mybir
import concourse.tile as tile
from concourse import bass_utils

AF = mybir.ActivationFunctionType
OP = mybir.AluOpType
PM = mybir.MatmulPerfMode
F32 = mybir.dt.float32
BF16 = mybir.dt.bfloat16
FP8 = mybir.dt.float8e4
U8 = mybir.dt.uint8

E, S, F, H, C, L = 64, 128, 640, 256, 32, 20
EPS = 1e-5
NCORES = 8
B = E * S              # 8192
BC = B // NCORES       # 1024 batch rows per core
NCH = 2                # chunks per core
CB = BC // NCH         # 512 batch rows per chunk (PSUM-bank friendly)

SH = 8.0               # h prescale into fp8 (keeps |8h|>=0.016 out of the
                       # e4m3 subnormal range the PE flushes to zero)
SW = 64.0              # W_hh prescale into fp8
SP = SH * SW           # gate-psum scale (512)

# const-vector columns (packed [128, NV], one column per per-partition vec)
CV_S1, CV_T1 = 0, 2
CV_GIBRZ = 4           # 512*(b_ih+b_hh) r,z (4 cols)
CV_GIBN = 8            # 512*b_ih n-part (2 cols)
CV_S2Z, CV_T2Z = 10, 12  # s2, -t2 (or t2) per m (2 each)
CV_C3 = 14             # conv const per m (2 cols)
CV_BMU, CV_A1, CV_A3 = 16, 17, 18
CV_BHN = 19            # SP*b_hh n-part (2 cols, zero in practice)
NV = 21

# bf16 weight pack (wp) column layout
WB_LIN = 0             # W_lin.T k-major: 5*256 = 1280
WB_IH = 1280           # W_ih.T k-major: 2*768 = 1536
WB_W1 = 2816           # conv a2-branch  (k-major): 2*256 = 512
WB_W2 = 3328           # conv relu-branch (k-major): 2*256 = 512
WB_MU = 3840           # W_mu.T k-major: 2*32 = 64
WB_ID = 3904           # identity 128
WB_COLS = 4032

_CACHE: dict = {}


def build_program(bhn_zero=True, zr_1op=True):
    nc = bacc.Bacc("TRN2", target_bir_lowering=False, debug=False)

    xT_h = nc.dram_tensor("xT", [128, 5 * BC], BF16, kind="ExternalInput")
    wp_h = nc.dram_tensor("wp", [128, WB_COLS], BF16, kind="ExternalInput")
    # fp8 weights ship as raw bytes; bitcast to fp8 at the use site
    wp8_h = nc.dram_tensor("wp8", [128, 1536], U8, kind="ExternalInput")
    cv_h = nc.dram_tensor("cv", [128, NV], F32, kind="ExternalInput")
    out_h = nc.dram_tensor("out", [BC, L * C], F32, kind="ExternalOutput")

    with tile.TileContext(nc) as tc:
        with (
            tc.tile_pool(name="consts", bufs=1) as cpool,
            tc.tile_pool(name="gi", bufs=1) as gip,
            tc.tile_pool(name="hp", bufs=2) as hp,
            tc.tile_pool(name="gates", bufs=3) as gp,
            tc.tile_pool(name="s24", bufs=2) as sp,
            tc.tile_pool(name="ps2", bufs=3, space="PSUM") as ps2,
            tc.tile_pool(name="ps1", bufs=2, space="PSUM") as ps1,
        ):
            cvt = cpool.tile([128, NV], F32, tag="cv")
            nc.sync.dma_start(cvt[:], cv_h[:])
            wpt_a = cpool.tile([128, WB_W1], BF16, tag="wpta")
            nc.sync.dma_start(wpt_a[:], wp_h[:, :WB_W1])
            wpt_b = cpool.tile([128, WB_COLS - WB_W1], BF16, tag="wptb")
            wp8t_u8 = cpool.tile([128, 1536], U8, tag="wp8t")
            wlin_t = wpt_a[:, WB_LIN : WB_LIN + 1280]
            wih_t = wpt_a[:, WB_IH : WB_IH + 1536]
            w1_t = wpt_b[:, WB_W1 - WB_W1 : WB_W2 - WB_W1]
            w2_t = wpt_b[:, WB_W2 - WB_W1 : WB_MU - WB_W1]
            wmu_t = wpt_b[:, WB_MU - WB_W1 : WB_ID - WB_W1]
            idtb = wpt_b[:, WB_ID - WB_W1 : WB_COLS - WB_W1]
            whh_dr = wp8t_u8[:, :].bitcast(FP8).rearrange("p (k r) -> p k r", k=2)

            def pp(col):
                return cvt[:, col : col + 1]

            def drmov(tile_):  # DR moving view of a [128, 2*CB] fp8 tile
                return tile_[:].rearrange("p (k b) -> p k b", k=2)

            # ---- stage 1: x1 = prelu(bn(x @ W_lin.T)); gi = x1 @ W_ih.T ----
            # all gi parts stored x512 (r/z feed PSUM preloads; tanh's free
            # scale absorbs 1/512 for the n part)
            girz = [None] * NCH
            gin = [None] * NCH
            with tc.tile_pool(name="stage1", bufs=1) as xp:
                xts = []
                for k in range(5):
                    xk = xp.tile([128, BC], BF16, tag=f"xT{k}", name=f"xT{k}")
                    nc.sync.dma_start(xk[:], xT_h[:, k * BC : (k + 1) * BC])
                    xts.append(xk)
                nc.sync.dma_start(wpt_b[:], wp_h[:, WB_W1:])
                nc.sync.dma_start(wp8t_u8[:], wp8_h[:])
                for c in range(NCH):
                    ps_x1 = ps2.tile([128, 2 * CB], F32, tag="g")
                    for k in range(5):
                        for m in range(2):
                            nc.tensor.matmul(
                                ps_x1[:, m * CB : (m + 1) * CB],
                                wlin_t[:, k * H + m * 128 : k * H + (m + 1) * 128],
                                xts[k][:, c * CB : (c + 1) * CB],
                                start=(k == 0),
                                stop=(k == 4),
                            )
                    x1p = xp.tile([128, 2 * CB], BF16, tag=f"x1p{c}")
                    for m in range(2):
                        nc.scalar.activation(
                            x1p[:, m * CB : (m + 1) * CB],
                            ps_x1[:, m * CB : (m + 1) * CB],
                            AF.Prelu,
                            bias=pp(CV_T1 + m),
                            scale=pp(CV_S1 + m),
                            alpha=pp(CV_A1),
                        )
                    girz[c] = gip.tile(
                        [128, 4 * CB], BF16, tag=f"girz{c}", name=f"girz{c}"
                    )
                    gin[c] = gip.tile(
                        [128, 2 * CB], BF16, tag=f"gin{c}", name=f"gin{c}"
                    )
                    for m in range(6):
                        ps_gi = ps1.tile([128, CB], F32, tag="p")
                        for k in range(2):
                            nc.tensor.matmul(
                                ps_gi[:],
                                wih_t[:, k * 3 * H + m * 128 : k * 3 * H + (m + 1) * 128],
                                x1p[:, k * CB : (k + 1) * CB],
                                start=(k == 0),
                                stop=(k == 1),
                            )
                        dstv = (
                            girz[c][:, m * CB : (m + 1) * CB]
                            if m < 4
                            else gin[c][:, (m - 4) * CB : (m - 3) * CB]
                        )
                        nc.scalar.activation(
                            dstv,
                            ps_gi[:],
                            AF.Identity,
                            bias=pp((CV_GIBRZ + m) if m < 4 else (CV_GIBN + m - 4)),
                            scale=SP,
                        )

            hs = [None] * NCH       # h (bf16)
            h8s = [None] * NCH      # 8*h (fp8) for the DR gate matmuls
            zrs = [None] * NCH      # relu-branch conv operand (bf16)
            y2s = [None] * NCH
            packs = [None] * NCH

            # step-0 h8 is just zeros; running the gate matmuls against it
            # keeps the PE warm and the results exact
            for c in range(NCH):
                h80 = hp.tile([128, 2 * CB], FP8, tag=f"h8{c}", name=f"h8{c}")
                nc.vector.memset(h80[:], 0.0)
                h8s[c] = h80

            def emit_stage2(c):
                h8 = hp.tile([128, 2 * CB], FP8, tag=f"h8{c}", name=f"h8{c}")
                nc.vector.tensor_scalar_mul(h8[:], hs[c][:], SH)
                h8s[c] = h8
                zr = gp.tile([128, 2 * CB], BF16, tag="zr", name=f"zr{c}")
                for m in range(2):
                    nc.vector.tensor_scalar(
                        zr[:, m * CB : (m + 1) * CB],
                        hs[c][:, m * CB : (m + 1) * CB],
                        pp(CV_S2Z + m),
                        pp(CV_T2Z + m),
                        op0=OP.mult,
                        op1=OP.max if zr_1op else OP.add,
                    )
                if not zr_1op:
                    nc.vector.tensor_scalar_max(zr[:], zr[:], 0.0)
                zrs[c] = zr

            def emit_conv(c):
                """conv psum for the latest h: W1@h + W2@zr (c3 rides the
                prelu3 bias)"""
                ps_cv = ps2.tile([128, 2 * CB], F32, tag="g", name=f"pscv{c}")
                for m in range(2):
                    for k in range(2):
                        nc.tensor.matmul(
                            ps_cv[:, m * CB : (m + 1) * CB],
                            w1_t[:, k * H + m * 128 : k * H + (m + 1) * 128],
                            hs[c][:, k * CB : (k + 1) * CB],
                            start=(k == 0),
                            stop=False,
                        )
                        nc.tensor.matmul(
                            ps_cv[:, m * CB : (m + 1) * CB],
                            w2_t[:, k * H + m * 128 : k * H + (m + 1) * 128],
                            zrs[c][:, k * CB : (k + 1) * CB],
                            start=False,
                            stop=(k == 1),
                        )
                return ps_cv

            def emit_prelu3(ps_cv, c):
                y2 = gp.tile([128, 2 * CB], BF16, tag="y2", name=f"y2_{c}")
                for m in range(2):
                    nc.scalar.activation(
                        y2[:, m * CB : (m + 1) * CB],
                        ps_cv[:, m * CB : (m + 1) * CB],
                        AF.Prelu,
                        bias=pp(CV_C3 + m),
                        alpha=pp(CV_A3),
                    )
                y2s[c] = y2

            def emit_wmu(t, c):
                j = t % 4
                if j == 0:
                    packs[c] = ps1.tile([128, CB], F32, tag="p", name=f"pack{c}")
                for k in range(2):
                    nc.tensor.matmul(
                        packs[c][32 * j : 32 * (j + 1), :],
                        wmu_t[:, k * C : (k + 1) * C],
                        y2s[c][:, k * CB : (k + 1) * CB],
                        start=(k == 0),
                        stop=(k == 1),
                        tile_position=(0, 32 * j),
                    )

            def emit_pack_out(t, c):
                if t % 4 != 3 or packs[c] is None:
                    return
                ps_pack = packs[c]
                pk = sp.tile([128, CB], F32, tag="pk", name=f"pk{c}")
                nc.scalar.activation(pk[:], ps_pack[:], AF.Identity, bias=pp(CV_BMU))
                tr = sp.tile([128, CB], F32, tag="tr", name=f"tr{c}")
                nc.vector.transpose(tr[:], pk[:])
                l4 = t // 4
                dst = out_h[:].rearrange(
                    "(c bh bl) (l4 li cc) -> c l4 li bl bh cc",
                    c=NCH, bh=CB // 32, bl=32, l4=L // 4, li=4, cc=C,
                )
                for li in range(4):
                    nc.sync.dma_start(
                        dst[c, l4, li],
                        tr[32 * li : 32 * (li + 1), :].rearrange(
                            "p (bh cc) -> p bh cc", bh=CB // 32
                        ),
                    )

            def emit_gate_mms_chunk(c, psr, psz, psn):
                for g, ps in ((0, psr), (1, psz)):
                    for m in range(2):
                        nc.tensor.matmul(
                            ps[:, m * CB : (m + 1) * CB],
                            idtb,
                            girz[c][:, (2 * g + m) * CB : (2 * g + m + 1) * CB],
                            start=True,
                            stop=False,
                        )
                for g, ps in ((0, psr), (1, psz), (2, psn)):
                    for m in range(2):
                        row = 2 * g + m
                        nc.tensor.matmul(
                            ps[:, m * CB : (m + 1) * CB],
                            whh_dr[:, :, row * 128 : (row + 1) * 128],
                            drmov(h8s[c]),
                            start=(g == 2),
                            stop=True,
                            perf_mode=PM.DoubleRow,
                        )

            # ---- GRU steps t = 0..L-1 (uniform; t=0 skips dead elementwise) ----
            for t in range(L):
                psr, psz, psn = [], [], []
                for c in range(NCH):
                    psr.append(ps2.tile([128, 2 * CB], F32, tag="g", name=f"psr{c}"))
                    psz.append(ps2.tile([128, 2 * CB], F32, tag="g", name=f"psz{c}"))
                    psn.append(ps2.tile([128, 2 * CB], F32, tag="g", name=f"psn{c}"))
                for c in range(NCH):
                    emit_gate_mms_chunk(c, psr[c], psz[c], psn[c])
                pscs = [None] * NCH
                if t >= 1:
                    for c in range(NCH):
                        pscs[c] = emit_conv(c)      # conv of h_t (prev state)
                if t >= 2:
                    for c in range(NCH):
                        emit_wmu(t - 2, c)
                for c in range(NCH):
                    z_sb = gp.tile([128, 2 * CB], BF16, tag="z", name=f"z{c}")
                    nc.scalar.activation(z_sb[:], psz[c][:], AF.Sigmoid, scale=1.0 / SP)
                    if t == 0:
                        # psn = W@0 = 0 -> n = tanh(gin); h1 = n - z*n
                        n_sb = gp.tile([128, 2 * CB], BF16, tag="n", name=f"n{c}")
                        nc.scalar.activation(n_sb[:], gin[c][:], AF.Tanh, scale=1.0 / SP)
                        v_sb = gp.tile([128, 2 * CB], BF16, tag="v", name=f"v{c}")
                        nc.vector.tensor_tensor(v_sb[:], z_sb[:], n_sb[:], OP.mult)
                        h = hp.tile([128, 2 * CB], BF16, tag=f"h{c}", name=f"h{c}")
                        nc.vector.tensor_tensor(h[:], n_sb[:], v_sb[:], OP.subtract)
                        hs[c] = h
                        continue
                    r_sb = gp.tile([128, 2 * CB], BF16, tag="r", name=f"r{c}")
                    nc.scalar.activation(r_sb[:], psr[c][:], AF.Sigmoid, scale=1.0 / SP)
                    t_sb = gp.tile([128, 2 * CB], BF16, tag="t", name=f"t{c}")
                    nc.vector.tensor_tensor(t_sb[:], psn[c][:], r_sb[:], OP.mult)
                    a_sb = gp.tile([128, 2 * CB], BF16, tag="a", name=f"a{c}")
                    if bhn_zero:
                        nc.vector.tensor_tensor(a_sb[:], t_sb[:], gin[c][:], OP.add)
                    else:
                        # a*SP = t + gin + SP*bhn*r
                        tmp = gp.tile([128, 2 * CB], BF16, tag="tb", name=f"tb{c}")
                        for m in range(2):
                            nc.vector.tensor_scalar_mul(
                                tmp[:, m * CB : (m + 1) * CB],
                                r_sb[:, m * CB : (m + 1) * CB],
                                pp(CV_BHN + m),
                            )
                        nc.vector.tensor_tensor(tmp[:], t_sb[:], tmp[:], OP.add)
                        nc.vector.tensor_tensor(a_sb[:], tmp[:], gin[c][:], OP.add)
                    n_sb = gp.tile([128, 2 * CB], BF16, tag="n", name=f"n{c}")
                    nc.scalar.activation(n_sb[:], a_sb[:], AF.Tanh, scale=1.0 / SP)
                    u_sb = gp.tile([128, 2 * CB], BF16, tag="u", name=f"u{c}")
                    nc.vector.tensor_tensor(u_sb[:], hs[c][:], n_sb[:], OP.subtract)
                    v_sb = gp.tile([128, 2 * CB], BF16, tag="v", name=f"v{c}")
                    nc.vector.tensor_tensor(v_sb[:], z_sb[:], u_sb[:], OP.mult)
                    h = hp.tile([128, 2 * CB], BF16, tag=f"h{c}", name=f"h{c}")
                    nc.vector.tensor_tensor(h[:], n_sb[:], v_sb[:], OP.add)
                    hs[c] = h
                    if t >= 1 and pscs[c] is not None:
                        emit_prelu3(pscs[c], c)
                for c in range(NCH):
                    emit_stage2(c)
                if t >= 2:
                    for c in range(NCH):
                        emit_pack_out(t - 2, c)
            # epilogue: conv/head for the last two states
            pscs = [emit_conv(c) for c in range(NCH)]
            for c in range(NCH):
                emit_wmu(L - 2, c)
            for c in range(NCH):
                emit_prelu3(pscs[c], c)
            for c in range(NCH):
                emit_pack_out(L - 2, c)
            for c in range(NCH):
                emit_wmu(L - 1, c)
            for c in range(NCH):
                emit_pack_out(L - 1, c)

    nc.compile()
    return nc


def _to_fp8(a):
    return np.clip(np.asarray(a, np.float32), -240.0, 240.0).astype(
        ml_dtypes.float8_e4m3
    )


def _prep_inputs(inputs):
    f32 = np.float32
    bf = ml_dtypes.bfloat16
    x = np.ascontiguousarray(np.asarray(inputs["h_w_action"], f32).reshape(B, F))
    W_lin = np.asarray(inputs["W_lin"], f32)
    b_lin = np.asarray(inputs["b_lin"], f32)
    W_ih = np.asarray(inputs["W_ih"], f32)
    W_hh = np.asarray(inputs["W_hh"], f32)
    b_ih = np.asarray(inputs["b_ih"], f32)
    b_hh = np.asarray(inputs["b_hh"], f32)
    Wc = np.asarray(inputs["Wc"], f32)
    bc = np.asarray(inputs["bc"], f32)
    W_mu = np.asarray(inputs["W_mu"], f32)
    b_mu = np.asarray(inputs["b_mu"], f32)
    a1 = f32(np.asarray(inputs["a1"]).reshape(-1)[0])
    a2 = f32(np.asarray(inputs["a2"]).reshape(-1)[0])
    a3 = f32(np.asarray(inputs["a3"]).reshape(-1)[0])

    def bnfold(g, beta, m, v):
        s = np.asarray(g, f32) / np.sqrt(np.asarray(v, f32) + EPS)
        return s, np.asarray(beta, f32) - np.asarray(m, f32) * s

    s1, t1 = bnfold(inputs["g1"], inputs["beta1"], inputs["m1"], inputs["v1"])
    s2, t2 = bnfold(inputs["g2"], inputs["beta2"], inputs["m2"], inputs["v2"])
    s3, t3 = bnfold(inputs["g3"], inputs["beta3"], inputs["m3"], inputs["v3"])
    t1 = t1 + s1 * b_lin
    t3 = t3 + s3 * bc
    gib = b_ih.copy()
    gib[: 2 * H] += b_hh[: 2 * H]
    bhn = b_hh[2 * H :]
    bhn_zero = bool(np.all(bhn == 0))
    zr_1op = bool(np.all(s2 > 0))

    # conv folding: conv_in = a2*s2*h + (1-a2)*zr + const
    #   zr_1op: zr = max(s2*h, -t2), const = t2
    #   else:   zr = relu(s2*h + t2), const = a2*t2
    W1 = a2 * (s3[:, None] * Wc * s2[None, :])
    W2 = (1.0 - a2) * (s3[:, None] * Wc)
    c3 = s3 * (Wc @ (t2 if zr_1op else a2 * t2)) + t3

    cv = np.zeros((128, NV), f32)
    for col, vec in ((CV_S1, s1), (CV_T1, t1)):
        cv[:, col] = vec[:128]
        cv[:, col + 1] = vec[128:]
    for m in range(4):
        cv[:, CV_GIBRZ + m] = SP * gib[m * 128 : (m + 1) * 128]
    for m in range(2):
        cv[:, CV_GIBN + m] = SP * gib[(4 + m) * 128 : (5 + m) * 128]
        cv[:, CV_BHN + m] = SP * bhn[m * 128 : (m + 1) * 128]
        cv[:, CV_S2Z + m] = s2[m * 128 : (m + 1) * 128]
        cv[:, CV_T2Z + m] = (-t2 if zr_1op else t2)[m * 128 : (m + 1) * 128]
        cv[:, CV_C3 + m] = c3[m * 128 : (m + 1) * 128]
    cv[:, CV_BMU] = np.tile(b_mu, 4)
    cv[:, CV_A1] = a1
    cv[:, CV_A3] = a3

    def kmaj(a, kt):  # [kt*128, n] -> [128, kt*n] k-tile-major columns
        n = a.shape[1]
        return a.reshape(kt, 128, n).transpose(1, 0, 2).reshape(128, kt * n)

    wp = np.concatenate(
        [
            kmaj(np.ascontiguousarray(W_lin.T), 5),
            kmaj(np.ascontiguousarray(W_ih.T), 2),
            kmaj(np.ascontiguousarray(W1.T), 2),
            kmaj(np.ascontiguousarray(W2.T), 2),
            kmaj(np.ascontiguousarray(W_mu.T), 2),
            np.eye(128, dtype=f32),
        ],
        axis=1,
    ).astype(bf)
    wp8 = kmaj(np.ascontiguousarray(SW * W_hh.T), 2)
    shared = {
        "wp": np.ascontiguousarray(wp),
        "wp8": np.ascontiguousarray(_to_fp8(wp8)).view(np.uint8),
        "cv": cv,
    }
    in_maps = []
    for i in range(NCORES):
        m = dict(shared)
        xtc = np.ascontiguousarray(x[i * BC : (i + 1) * BC, :].T)  # [640, BC]
        m["xT"] = np.ascontiguousarray(kmaj(xtc, 5).astype(bf))
        in_maps.append(m)
    return in_maps, bhn_zero, zr_1op


def kernel(**inputs) -> np.ndarray:
    in_maps, bhn_zero, zr_1op = _prep_inputs(inputs)
    key = ("nc", bhn_zero, zr_1op)
    if key not in _CACHE:
        _CACHE[key] = build_program(bhn_zero, zr_1op)
    nc = _CACHE[key]
    _CACHE["last"] = nc
    res = bass_utils.run_bass_kernel_spmd(nc, in_maps, core_ids=list(range(NCORES)))
    outs = [np.asarray(r["out"], np.float32) for r in res.results]
    return np.concatenate(outs, axis=0).reshape(E, S, L, C)
